# revision 2
# baseline (speedup 1.0000x reference)
"""Trainium2 Bass kernel for BaseFisheyeLSSTransform (BEV pooling).

Strategy (output-sharded uniform SPMD over 8 NeuronCores):
- Host (cheap, index-only math): replicate the reference voxelization on
  jax-cpu fp32 to get each kept point's (batch, x-row, cy, 1/count). Points
  are grouped per output x-row, ordered by source memory index, merged into
  multi-row spans, and encoded as indirect-DMA descriptors (class-2 spans
  of <=2 rows, class-8 spans of 3..8 rows).
- Device: each core owns a balanced subset of x-rows of one batch. Per
  instruction: one indirect DMA gathers 128 descriptors from x[b]
  ([566400, 80] fp32) into SBUF [128, L*80]. Per column-block l a single
  fused DVE op builds M = (iota360 == vid)*invcnt, and partition-sliced
  matmuls accumulate PSUM[row] += X_l^T @ M_l ([80, 360] per x-row).
  Closed rows are copied to an SBUF slab and flushed to DRAM [80, NSLOTS*360].
- The instruction structure is identical on all cores (SPMD); all per-core
  variation is carried in data slabs (descriptor starts, vid, invcnt).
- Host assembles the final [2, 80, 360, 360] from the 8 slabs (pure unshard:
  each x-row is produced by exactly one core; empty rows are zeros).
"""
import sys

sys.path.insert(0, "/opt/trn_rl_repo")

import numpy as np

B, N, C = 2, 4, 80
FH, FW, D = 40, 60, 59
NX, NY = 360, 360
PB = N * D * FH * FW  # 566400 rows per batch slice of x
GAP_TOL = 2
P = 128
QUANT = 64
FLUSH_WINDOWS = 16


# ---------------------------------------------------------------- schedule


def _geometry(camera2lidar_rots, camera2lidar_trans):
    import jax
    import jax.numpy as jnp

    cpu = jax.devices("cpu")[0]
    with jax.default_device(cpu):
        DX = jnp.array([0.3, 0.3, 8.0], dtype=jnp.float32)
        ORIGIN = jnp.array([-54.0, -54.0, -5.0], dtype=jnp.float32)
        ds = jnp.arange(1.0, 60.0, 1.0, dtype=jnp.float32)
        az = jnp.linspace(-1.92, 1.92, FW, dtype=jnp.float32)
        el = jnp.linspace(-0.61, 0.61, FH, dtype=jnp.float32)
        d_, e_, a_ = ds[:, None, None], el[None, :, None], az[None, None, :]
        xs = d_ * jnp.cos(e_) * jnp.sin(a_)
        ys = jnp.broadcast_to(d_ * jnp.sin(e_), (D, FH, FW))
        zs = d_ * jnp.cos(e_) * jnp.cos(a_)
        fr = jnp.stack([xs, ys, zs], axis=-1)
        geom = jnp.einsum("bnij,dhwj->bndhwi", camera2lidar_rots, fr)
        geom = geom + camera2lidar_trans[:, :, None, None, None, :]
        coords = np.asarray(((geom - ORIGIN) / DX).astype(jnp.int32))
    kept = (
        (coords[..., 0] >= 0) & (coords[..., 0] < NX)
        & (coords[..., 1] >= 0) & (coords[..., 1] < NY)
        & (coords[..., 2] >= 0) & (coords[..., 2] < 1)
    )
    return coords, kept


def _build_rows(coords, kept):
    rows = {}
    for b in range(B):
        k = kept[b].reshape(-1)
        cx = coords[b, ..., 0].reshape(-1)
        cy = coords[b, ..., 1].reshape(-1)
        pts = np.flatnonzero(k)
        lin = cx[pts].astype(np.int64) * NY + cy[pts]
        cnt = np.bincount(lin, minlength=NX * NY)
        order = np.lexsort((pts, cx[pts]))
        sp = pts[order]
        sx = cx[pts][order]
        sy = cy[pts][order]
        w = (1.0 / np.maximum(cnt[lin[order]], 1)).astype(np.float32)
        new = np.ones(sp.size, bool)
        new[1:] = (np.diff(sx) != 0) | (np.diff(sp) > (GAP_TOL + 1))
        starts = np.flatnonzero(new)
        ends = np.append(starts[1:], sp.size)
        for s, e in zip(starts, ends):
            key = (b, int(sx[s]))
            if key not in rows:
                rows[key] = {2: [], 8: []}
            lane = {int(sp[i]): (int(sy[i]), float(w[i])) for i in range(s, e)}
            lo, hi = int(sp[s]), int(sp[e - 1])
            base = lo
            while base <= hi:
                span = hi - base + 1
                L = 2 if span <= 2 else 8
                start = max(0, min(base, PB - L))
                vids, ws = [], []
                for l in range(L):
                    r = start + l
                    if r in lane and r >= base:
                        vids.append(lane[r][0])
                        ws.append(lane[r][1])
                    else:
                        vids.append(-1)
                        ws.append(0.0)
                rows[key][L].append((start, vids, ws))
                base = start + L
    return rows


def _assign_cores(rows):
    cores = [[] for _ in range(8)]
    load = [0] * 8
    for b in range(B):
        keys = [k for k in rows if k[0] == b]
        keys.sort(key=lambda k: -(len(rows[k][2]) + len(rows[k][8])))
        for k in keys:
            cost = len(rows[k][2]) + len(rows[k][8])
            ci = min(range(4 * b, 4 * b + 4), key=lambda i: load[i])
            cores[ci].append(k)
            load[ci] += cost
    return cores, load


def _ceil(a, b):
    return -(-a // b)


def _build_uniform_schedule(rows, cores):
    core_rows = []
    NW = 0
    for ci in range(8):
        ks = sorted(cores[ci], key=lambda k: -(len(rows[k][2]) + len(rows[k][8])))
        core_rows.append(ks)
        NW = max(NW, len(ks))

    q2 = np.zeros(NW, np.int64)
    q8 = np.zeros(NW, np.int64)
    for ci in range(8):
        for w, key in enumerate(core_rows[ci]):
            q2[w] = max(q2[w], _ceil(len(rows[key][2]), QUANT))
            q8[w] = max(q8[w], _ceil(len(rows[key][8]), QUANT))

    def stream_instrs(qcounts):
        # Lane masking on lhsT makes any slice legal; pack maximally.
        NQ_PER_INSTR = P // QUANT
        instrs = []
        cur = []
        used = 0
        for w in range(NW):
            need = int(qcounts[w])
            while need > 0:
                take = min(NQ_PER_INSTR - used, need)
                cur.append((w, used * QUANT, (used + take) * QUANT))
                used += take
                need -= take
                if used == NQ_PER_INSTR:
                    instrs.append(cur)
                    cur = []
                    used = 0
        if cur:
            instrs.append(cur)
        return instrs

    i2 = stream_instrs(q2)
    i8 = stream_instrs(q8)
    tagged = [(min(t[0] for t in ins), 0, j, 2, ins) for j, ins in enumerate(i2)]
    tagged += [(min(t[0] for t in ins), 1, j, 8, ins) for j, ins in enumerate(i8)]
    tagged.sort(key=lambda t: (t[0], t[1], t[2]))

    struct = []
    cb0 = 0
    first_seen = {}
    last_seen = {}
    for ii, (_, _, _, cls, ins) in enumerate(tagged):
        tasks = [[l, lo, hi, w, False, False] for (w, lo, hi) in ins
                 for l in range(cls)]
        for (w, lo, hi) in ins:
            if w not in first_seen:
                first_seen[w] = ii
            last_seen[w] = ii
        struct.append(dict(cls=cls, cb0=cb0, tasks=tasks, copies_after=[]))
        cb0 += cls
    NCB = cb0
    NINSTR = len(struct)

    started = set()
    for rec in struct:
        for t in rec["tasks"]:
            if t[3] not in started:
                started.add(t[3])
                t[4] = True
    for w, ii in last_seen.items():
        rec = struct[ii]
        lastj = max(j for j, t in enumerate(rec["tasks"]) if t[3] == w)
        rec["tasks"][lastj][5] = True
    for rec in struct:
        rec["tasks"] = [tuple(t) for t in rec["tasks"]]
    for w, ii in last_seen.items():
        struct[ii]["copies_after"].append(w)
    NSLOTS = NW
    nblocks = _ceil(NSLOTS, FLUSH_WINDOWS)
    for k in range(nblocks):
        ws = [w for w in range(k * FLUSH_WINDOWS,
                               min((k + 1) * FLUSH_WINDOWS, NSLOTS))
              if w in last_seen]
        pos = max(last_seen[w] for w in ws) if ws else 0
        struct[pos].setdefault("flushes", []).append(k)

    per_core = []
    for ci in range(8):
        desc = np.zeros((P, NINSTR), np.int32)
        vid = np.full((P, NCB), -1.0, np.float32)
        invpc = np.zeros((P, NCB), np.float32)
        slot_rows = [None] * NSLOTS
        for w, key in enumerate(core_rows[ci]):
            slot_rows[w] = key
        cursor = {}
        for ii, rec in enumerate(struct):
            cls = rec["cls"]
            seen = set()
            for (l, lo, hi, w, st, sp_) in rec["tasks"]:
                if (w, lo) in seen:
                    continue
                seen.add((w, lo))
                if w >= len(core_rows[ci]):
                    continue
                key = core_rows[ci][w]
                dlist = rows[key][cls]
                cur = cursor.get((cls, w), 0)
                chunk = dlist[cur : cur + (hi - lo)]
                cursor[(cls, w)] = cur + (hi - lo)
                for j, (start, vids, ws_) in enumerate(chunk):
                    p_ = lo + j
                    desc[p_, ii] = start
                    for l2 in range(cls):
                        vid[p_, rec["cb0"] + l2] = vids[l2]
                        invpc[p_, rec["cb0"] + l2] = ws_[l2]
        per_core.append(dict(desc=desc, vid=vid, invpc=invpc,
                             slot_rows=slot_rows))

    return dict(struct=struct, NSLOTS=NSLOTS, NINSTR=NINSTR, NCB=NCB,
                per_core=per_core, nblocks=nblocks)


def build_schedule(camera2lidar_rots, camera2lidar_trans):
    coords, kept = _geometry(camera2lidar_rots, camera2lidar_trans)
    rows = _build_rows(coords, kept)
    cores, load = _assign_cores(rows)
    sched = _build_uniform_schedule(rows, cores)
    sched["load"] = load
    return sched


# ---------------------------------------------------------------- device


def mask_bank():
    combos = [(lo, hi) for lo in (0, 32, 64, 96) for hi in (32, 64, 96, 128)
              if lo < hi and not (lo == 0 and hi == 128)]
    mb = np.zeros((P, len(combos)), np.float32)
    for i, (lo, hi) in enumerate(combos):
        mb[lo:hi, i] = 1.0
    return mb


def build_program(sched):
    import concourse.bacc as bacc
    import concourse.bass as bass
    import concourse.mybir as mybir
    import concourse.tile as tile

    f32, i32 = mybir.dt.float32, mybir.dt.int32
    NINSTR, NCB, NSLOTS = sched["NINSTR"], sched["NCB"], sched["NSLOTS"]

    MASK_COMBOS = [(lo, hi) for lo in (0, 32, 64, 96) for hi in (32, 64, 96, 128)
                   if lo < hi and not (lo == 0 and hi == 128)]

    nc = bacc.Bacc(None)
    xb = nc.declare_dram_parameter("xb", [PB, C], f32, isOutput=False)
    maskb_d = nc.declare_dram_parameter("maskb", [P, len(MASK_COMBOS)], f32,
                                        isOutput=False)
    desc_d = nc.declare_dram_parameter("desc", [P, NINSTR], i32, isOutput=False)
    vid_d = nc.declare_dram_parameter("vid", [P, NCB], f32, isOutput=False)
    invpc_d = nc.declare_dram_parameter("invpc", [P, NCB], f32, isOutput=False)
    iota_d = nc.declare_dram_parameter("iota", [P, NY], f32, isOutput=False)
    out_d = nc.declare_dram_parameter("out", [C, NSLOTS * NY], f32,
                                      isOutput=True)

    with tile.TileContext(nc) as tc:
        with (
            tc.tile_pool(name="const", bufs=1) as cpool,
            tc.tile_pool(name="g2", bufs=8) as g2pool,
            tc.tile_pool(name="g8", bufs=4) as g8pool,
            tc.tile_pool(name="m", bufs=8) as mpool,
            tc.tile_pool(name="psum", bufs=8, space="PSUM") as ppool,
            tc.tile_pool(name="slab", bufs=3) as slabpool,
        ):
            desc_t = cpool.tile([P, NINSTR], i32)
            vid_t = cpool.tile([P, NCB], f32)
            invpc_t = cpool.tile([P, NCB], f32)
            iota_t = cpool.tile([P, NY], f32)
            maskb_t = cpool.tile([P, len(MASK_COMBOS)], f32)
            nc.sync.dma_start(out=maskb_t[:], in_=maskb_d[:])
            masks = {c: maskb_t[:, i : i + 1] for i, c in enumerate(MASK_COMBOS)}
            nc.sync.dma_start(out=desc_t[:], in_=desc_d[:])
            nc.sync.dma_start(out=vid_t[:], in_=vid_d[:])
            nc.sync.dma_start(out=invpc_t[:], in_=invpc_d[:])
            nc.sync.dma_start(out=iota_t[:], in_=iota_d[:])

            wtiles = {}
            slabs = {}
            for ii, rec in enumerate(sched["struct"]):
                L = rec["cls"]
                pool = g2pool if L == 2 else g8pool
                g = pool.tile([P, L * C], f32, tag=f"g{L}")
                nc.gpsimd.indirect_dma_start(
                    out=g[:],
                    out_offset=None,
                    in_=xb[:],
                    in_offset=bass.IndirectOffsetOnAxis(
                        ap=desc_t[:, ii : ii + 1], axis=0
                    ),
                )
                Ms = {}
                for l in range(L):
                    col = rec["cb0"] + l
                    M = mpool.tile([P, NY], f32, tag="m")
                    # M = (iota == vid) * invcnt, fused on DVE
                    nc.vector.tensor_scalar(
                        out=M[:],
                        in0=iota_t[:],
                        scalar1=vid_t[:, col : col + 1],
                        scalar2=invpc_t[:, col : col + 1],
                        op0=mybir.AluOpType.is_equal,
                        op1=mybir.AluOpType.mult,
                    )
                    Ms[l] = M
                for (l, lo, hi, w, st, sp_) in rec["tasks"]:
                    if st:
                        wtiles[w] = ppool.tile([C, NY], f32, tag="w", name=f"w{w}")
                    if lo == 0 and hi == 128:
                        lhs = g[:, l * C : (l + 1) * C]
                    else:
                        # full-K matmul with lanes outside [lo,hi) zeroed on
                        # the 80-wide lhsT (partition-sliced matmuls that
                        # accumulate are an HW/compiler hazard).
                        xm = mpool.tile([P, C], f32, tag="xm", name="xm")
                        nc.vector.tensor_scalar_mul(
                            xm[:], g[:, l * C : (l + 1) * C], masks[(lo, hi)]
                        )
                        lhs = xm[:]
                    nc.tensor.matmul(
                        wtiles[w][:],
                        lhs,
                        Ms[l][:],
                        start=st,
                        stop=sp_,
                        skip_group_check=True,
                    )
                for w in rec["copies_after"]:
                    blk = w // FLUSH_WINDOWS
                    if blk not in slabs:
                        slabs[blk] = slabpool.tile(
                            [C, FLUSH_WINDOWS * NY], f32, tag="slab",
                            name=f"slab{blk}",
                        )
                    off = w % FLUSH_WINDOWS
                    nc.vector.tensor_copy(
                        slabs[blk][:, off * NY : (off + 1) * NY],
                        wtiles.pop(w)[:],
                    )
                for blk in rec.get("flushes", []):
                    w0 = blk * FLUSH_WINDOWS
                    w1 = min(w0 + FLUSH_WINDOWS, NSLOTS)
                    nc.sync.dma_start(
                        out=out_d[:, w0 * NY : w1 * NY],
                        in_=slabs.pop(blk)[:, : (w1 - w0) * NY],
                    )
    nc.compile()
    return nc


def make_in_maps(sched, x):
    iota = np.broadcast_to(
        np.arange(NY, dtype=np.float32)[None, :], (P, NY)
    ).copy()
    maskb = mask_bank()
    in_maps = []
    for ci in range(8):
        b = 0 if ci < 4 else 1
        pc = sched["per_core"][ci]
        in_maps.append(
            {
                "xb": np.ascontiguousarray(x[b].reshape(PB, C)),
                "desc": pc["desc"],
                "vid": pc["vid"],
                "invpc": pc["invpc"],
                "iota": iota,
                "maskb": maskb,
            }
        )
    return in_maps


def run_on_device(sched, x):
    from concourse.bass_utils import run_bass_kernel_spmd

    nc = build_program(sched)
    in_maps = make_in_maps(sched, x)
    res = run_bass_kernel_spmd(nc, in_maps, list(range(8)))
    return [res.results[ci]["out"] for ci in range(8)]


def assemble(slabs, sched):
    out = np.zeros((B, C, NX, NY), np.float32)
    for ci in range(8):
        pc = sched["per_core"][ci]
        slab = slabs[ci]
        for s, key in enumerate(pc["slot_rows"]):
            if key is None:
                continue
            b, xrow = key
            out[b, :, xrow, :] = slab[:, s * NY : (s + 1) * NY]
    return out


def kernel(x, camera2lidar_rots, camera2lidar_trans):
    x = np.asarray(x, dtype=np.float32)
    rots = np.asarray(camera2lidar_rots, dtype=np.float32)
    trans = np.asarray(camera2lidar_trans, dtype=np.float32)
    sched = build_schedule(rots, trans)
    slabs = run_on_device(sched, x)
    return assemble(slabs, sched)



# revision 3
# speedup vs baseline: 2.0597x; 2.0597x over previous
"""Trainium2 Bass kernel for BaseFisheyeLSSTransform (BEV pooling), v3.

Architecture (8-core SPMD, one program, per-core data tables):
- Host (index-only math): voxelize the frustum geometry on jax-cpu, find
  runs of kept points (consecutive source rows, same output x-row), encode
  them as indirect-DMA descriptors in two classes (L=1 single rows, L=4
  spans). Slots (b, x-row) are balanced across cores; per-(slot, class)
  descriptor counts are quantized and maxed over cores so all 8 cores share
  one instruction structure.
- Device: x is staged as one concatenated [2*566400, 80] fp16 tensor. Per
  instruction one indirect DMA gathers 128 descriptors into SBUF
  [128, L*80] fp16. Per (instruction, lane, slot) segment, a single fused
  DVE op builds M = (iota == vid) * invcnt in fp16 ([128, 360]); rows
  outside the segment (padding, other slots) carry vid = -1 so M is zero
  there and the fp16 matmul can contract over all 128 partitions without
  masking: PSUM[slot] += g_lane^T @ M. Closed slots are copied
  PSUM -> slab on the Scalar engine and flushed to DRAM in 16-slot blocks.
- Host assembles [2, 80, 360, 360] from the 8 slabs (pure unshard).
"""
import sys

sys.path.insert(0, "/opt/trn_rl_repo")

import numpy as np

B, N, C = 2, 4, 80
FH, FW, D = 40, 60, 59
NX, NY = 360, 360
PB = N * D * FH * FW  # 566400 rows per batch
P = 128
CLASSES = (1, 4)
QUANT = {1: 64, 4: 16}
FLUSH_WINDOWS = 16


# ---------------------------------------------------------------- host side


def _geometry(camera2lidar_rots, camera2lidar_trans):
    import jax
    import jax.numpy as jnp

    cpu = jax.devices("cpu")[0]
    with jax.default_device(cpu):
        DX = jnp.array([0.3, 0.3, 8.0], dtype=jnp.float32)
        ORIGIN = jnp.array([-54.0, -54.0, -5.0], dtype=jnp.float32)
        ds = jnp.arange(1.0, 60.0, 1.0, dtype=jnp.float32)
        az = jnp.linspace(-1.92, 1.92, FW, dtype=jnp.float32)
        el = jnp.linspace(-0.61, 0.61, FH, dtype=jnp.float32)
        d_, e_, a_ = ds[:, None, None], el[None, :, None], az[None, None, :]
        xs = d_ * jnp.cos(e_) * jnp.sin(a_)
        ys = jnp.broadcast_to(d_ * jnp.sin(e_), (D, FH, FW))
        zs = d_ * jnp.cos(e_) * jnp.cos(a_)
        fr = jnp.stack([xs, ys, zs], axis=-1)
        geom = jnp.einsum("bnij,dhwj->bndhwi", camera2lidar_rots, fr)
        geom = geom + camera2lidar_trans[:, :, None, None, None, :]
        coords = np.asarray(((geom - ORIGIN) / DX).astype(jnp.int32))
    kept = (
        (coords[..., 0] >= 0) & (coords[..., 0] < NX)
        & (coords[..., 1] >= 0) & (coords[..., 1] < NY)
        & (coords[..., 2] >= 0) & (coords[..., 2] < 1)
    )
    return coords, kept


def _build_runs(coords, kept):
    """rows[(b, xrow)] = {L: [(global_start, ys[L], ws[L])]}; ys=-1 pads."""
    Lmax = max(CLASSES)
    rows = {}
    for b in range(B):
        k = kept[b].reshape(-1)
        cx = coords[b, ..., 0].reshape(-1)
        cy = coords[b, ..., 1].reshape(-1)
        pts = np.flatnonzero(k)
        lin = cx[pts].astype(np.int64) * NY + cy[pts]
        cnt = np.bincount(lin, minlength=NX * NY)
        w_all = (1.0 / np.maximum(cnt, 1)).astype(np.float32)
        order = np.lexsort((pts, cx[pts]))
        sp = pts[order]
        sx = cx[pts][order]
        sy = cy[pts][order]
        sw = w_all[lin[order]]
        new = np.ones(sp.size, bool)
        new[1:] = (np.diff(sx) != 0) | (np.diff(sp) > Lmax)
        starts = np.flatnonzero(new)
        ends = np.append(starts[1:], sp.size)
        for s, e in zip(starts, ends):
            key = (b, int(sx[s]))
            if key not in rows:
                rows[key] = {L: [] for L in CLASSES}
            i = s
            while i < e:
                j = i
                base = int(sp[i])
                while j < e and int(sp[j]) - base < Lmax:
                    j += 1
                span = int(sp[j - 1]) - base + 1
                L = min(c for c in CLASSES if c >= span)
                gstart = b * PB + base
                gstart = min(gstart, 2 * PB - L)
                off = b * PB + base - gstart
                ys = np.full(L, -1.0, np.float32)
                ws = np.zeros(L, np.float32)
                for t in range(i, j):
                    ys[int(sp[t]) - base + off] = float(sy[t])
                    ws[int(sp[t]) - base + off] = float(sw[t])
                rows[key][L].append((gstart, ys, ws))
                i = j
    return rows


def _assign_slots(rows, n_cores=8):
    keys = sorted(rows, key=lambda k: -sum(len(v) for v in rows[k].values()))
    cores = [[] for _ in range(n_cores)]
    load = [0] * n_cores
    for k in keys:
        cost = sum(len(v) for v in rows[k].values())
        ci = min(range(n_cores), key=lambda i: load[i])
        cores[ci].append(k)
        load[ci] += cost
    return cores, load


def _ceil(a, b):
    return -(-a // b)


def build_schedule(camera2lidar_rots, camera2lidar_trans):
    coords, kept = _geometry(camera2lidar_rots, camera2lidar_trans)
    rows = _build_runs(coords, kept)
    cores, load = _assign_slots(rows)
    n_cores = len(cores)
    NSLOTS = max(len(c) for c in cores)

    nchunks = {}
    for cls in CLASSES:
        Q = QUANT[cls]
        for w in range(NSLOTS):
            m = 0
            for ci in range(n_cores):
                if w < len(cores[ci]):
                    m = max(m, _ceil(len(rows[cores[ci][w]][cls]), Q))
            nchunks[(cls, w)] = m

    instrs = []
    for cls in CLASSES:
        Q = QUANT[cls]
        npc = P // Q
        stream = []
        for w in range(NSLOTS):
            stream += [(w, q) for q in range(nchunks[(cls, w)])]
        for i0 in range(0, len(stream), npc):
            instrs.append(dict(cls=cls, chunks=stream[i0 : i0 + npc]))
    instrs.sort(key=lambda r: (min(c[0] for c in r["chunks"]),
                               r["cls"], r["chunks"][0][1]))
    NINSTR = len(instrs)

    # per-instruction chunk content per core
    content = []
    for rec in instrs:
        cls = rec["cls"]
        Q = QUANT[cls]
        per_chunk = []
        for (w, q) in rec["chunks"]:
            cores_dat = []
            for ci in range(n_cores):
                dlist = rows[cores[ci][w]][cls] if w < len(cores[ci]) else []
                chunk = dlist[q * Q : (q + 1) * Q]
                starts = np.zeros(Q, np.int64)
                ys = np.full((Q, cls), -1.0, np.float32)
                ws = np.zeros((Q, cls), np.float32)
                for j, (st, yy, ww) in enumerate(chunk):
                    starts[j] = st
                    ys[j] = yy
                    ws[j] = ww
                cores_dat.append((starts, ys, ws))
            per_chunk.append(cores_dat)
        content.append(per_chunk)

    # segments (uniform): (lane, k0, k1, w, col) — emitted if any core has
    # a real point; col indexes the vid/invpc tables
    NMBUILD = 0
    slot_last = {}
    slot_first = {}
    for ii, rec in enumerate(instrs):
        cls = rec["cls"]
        Q = QUANT[cls]
        segs = []
        for lane in range(cls):
            groups = []
            for k, (w, q) in enumerate(rec["chunks"]):
                if groups and groups[-1][0] == w and groups[-1][2] == k:
                    groups[-1][2] = k + 1
                else:
                    groups.append([w, k, k + 1])
            for (w, k0, k1) in groups:
                occ = any(
                    (content[ii][k][ci][1][:, lane] >= 0).any()
                    for k in range(k0, k1) for ci in range(n_cores)
                )
                if occ:
                    segs.append([lane, k0 * Q, k1 * Q, w, NMBUILD])
                    NMBUILD += 1
                    if w not in slot_first:
                        slot_first[w] = (ii, len(segs) - 1)
                    slot_last[w] = (ii, len(segs) - 1)
        rec["segments"] = segs

    # start/stop flags per segment; copies/flushes per instruction
    for ii, rec in enumerate(instrs):
        rec["copies_after"] = []
        rec["flushes"] = []
        for si, seg in enumerate(rec["segments"]):
            w = seg[3]
            seg.append(slot_first[w] == (ii, si))
            seg.append(slot_last[w] == (ii, si))
    for w, (ii, si) in slot_last.items():
        instrs[ii]["copies_after"].append(w)
    nblocks = _ceil(NSLOTS, FLUSH_WINDOWS)
    for blk in range(nblocks):
        ws = [w for w in range(blk * FLUSH_WINDOWS,
                               min((blk + 1) * FLUSH_WINDOWS, NSLOTS))
              if w in slot_last]
        pos = max(slot_last[w][0] for w in ws) if ws else 0
        instrs[pos]["flushes"].append(blk)

    # per-core tables
    per_core = []
    for ci in range(n_cores):
        desc = np.zeros((P, NINSTR), np.int32)
        vid = np.full((P, max(NMBUILD, 1)), -1.0, np.float32)
        invpc = np.zeros((P, max(NMBUILD, 1)), np.float32)
        for ii, rec in enumerate(instrs):
            cls = rec["cls"]
            Q = QUANT[cls]
            for k in range(len(rec["chunks"])):
                starts, ys, ws = content[ii][k][ci]
                desc[k * Q : (k + 1) * Q, ii] = starts
            for seg in rec["segments"]:
                lane, lo, hi, w, col = seg[:5]
                for k in range(lo // Q, hi // Q):
                    starts, ys, ws = content[ii][k][ci]
                    vid[k * Q : (k + 1) * Q, col] = ys[:, lane]
                    invpc[k * Q : (k + 1) * Q, col] = ws[:, lane]
        slot_rows = [cores[ci][w] if w < len(cores[ci]) else None
                     for w in range(NSLOTS)]
        per_core.append(dict(desc=desc, vid=vid, invpc=invpc,
                             slot_rows=slot_rows))

    return dict(instrs=instrs, NINSTR=NINSTR, NMBUILD=NMBUILD,
                NSLOTS=NSLOTS, per_core=per_core, load=load,
                nblocks=nblocks)


# ---------------------------------------------------------------- device


def build_program(sched):
    import concourse.bacc as bacc
    import concourse.bass as bass
    import concourse.mybir as mybir
    import concourse.tile as tile

    f32, f16 = mybir.dt.float32, mybir.dt.float16
    i32 = mybir.dt.int32
    NINSTR, NMBUILD = sched["NINSTR"], sched["NMBUILD"]
    NSLOTS = sched["NSLOTS"]

    nc = bacc.Bacc(None)
    xb = nc.declare_dram_parameter("xb", [2 * PB, C], f16, isOutput=False)
    desc_d = nc.declare_dram_parameter("desc", [P, NINSTR], i32,
                                       isOutput=False)
    vid_d = nc.declare_dram_parameter("vid", [P, NMBUILD], f32,
                                      isOutput=False)
    invpc_d = nc.declare_dram_parameter("invpc", [P, NMBUILD], f32,
                                        isOutput=False)
    iota_d = nc.declare_dram_parameter("iota", [P, NY], f16, isOutput=False)
    out_d = nc.declare_dram_parameter("out", [C, NSLOTS * NY], f32,
                                      isOutput=True)

    with tile.TileContext(nc) as tc:
        with (
            tc.tile_pool(name="const", bufs=1) as cpool,
            tc.tile_pool(name="g1", bufs=8) as g1pool,
            tc.tile_pool(name="g4", bufs=8) as g4pool,
            tc.tile_pool(name="m", bufs=8) as mpool,
            tc.tile_pool(name="psum", bufs=8, space="PSUM") as ppool,
            tc.tile_pool(name="slab", bufs=3) as slabpool,
        ):
            desc_t = cpool.tile([P, NINSTR], i32)
            vid_t = cpool.tile([P, NMBUILD], f32)
            invpc_t = cpool.tile([P, NMBUILD], f32)
            iota_t = cpool.tile([P, NY], f16)
            nc.sync.dma_start(out=desc_t[:], in_=desc_d[:])
            nc.sync.dma_start(out=vid_t[:], in_=vid_d[:])
            nc.sync.dma_start(out=invpc_t[:], in_=invpc_d[:])
            nc.sync.dma_start(out=iota_t[:], in_=iota_d[:])

            wtiles = {}
            slabs = {}
            for ii, rec in enumerate(sched["instrs"]):
                L = rec["cls"]
                pool = g1pool if L == 1 else g4pool
                g = pool.tile([P, L * C], f16, tag=f"g{L}")
                nc.gpsimd.indirect_dma_start(
                    out=g[:],
                    out_offset=None,
                    in_=xb[:],
                    in_offset=bass.IndirectOffsetOnAxis(
                        ap=desc_t[:, ii : ii + 1], axis=0
                    ),
                )
                for seg in rec["segments"]:
                    lane, lo, hi, w, col, st, sp_ = seg
                    M = mpool.tile([P, NY], f16, tag="m")
                    nc.vector.tensor_scalar(
                        out=M[:],
                        in0=iota_t[:],
                        scalar1=vid_t[:, col : col + 1],
                        scalar2=invpc_t[:, col : col + 1],
                        op0=mybir.AluOpType.is_equal,
                        op1=mybir.AluOpType.mult,
                    )
                    if st:
                        wtiles[w] = ppool.tile([C, NY], f32, tag="w",
                                               name=f"w{w}")
                    nc.tensor.matmul(
                        wtiles[w][:],
                        g[:, lane * C : (lane + 1) * C],
                        M[:],
                        start=st,
                        stop=sp_,
                        skip_group_check=True,
                    )
                for w in rec["copies_after"]:
                    blk = w // FLUSH_WINDOWS
                    if blk not in slabs:
                        slabs[blk] = slabpool.tile(
                            [C, FLUSH_WINDOWS * NY], f32, tag="slab",
                            name=f"slab{blk}",
                        )
                    off = w % FLUSH_WINDOWS
                    nc.scalar.activation(
                        out=slabs[blk][:, off * NY : (off + 1) * NY],
                        in_=wtiles.pop(w)[:],
                        func=mybir.ActivationFunctionType.Copy,
                    )
                for blk in rec["flushes"]:
                    w0 = blk * FLUSH_WINDOWS
                    w1 = min(w0 + FLUSH_WINDOWS, NSLOTS)
                    nc.sync.dma_start(
                        out=out_d[:, w0 * NY : w1 * NY],
                        in_=slabs.pop(blk)[:, : (w1 - w0) * NY],
                    )
    nc.compile()
    return nc


def make_in_maps(sched, x):
    xcat = np.ascontiguousarray(
        x.reshape(2 * PB, C)).astype(np.float16)
    iota = np.broadcast_to(
        np.arange(NY, dtype=np.float16)[None, :], (P, NY)
    ).copy()
    in_maps = []
    for ci in range(8):
        pc = sched["per_core"][ci]
        in_maps.append(
            {
                "xb": xcat,
                "desc": pc["desc"],
                "vid": pc["vid"],
                "invpc": pc["invpc"],
                "iota": iota,
            }
        )
    return in_maps


def assemble(slabs, sched):
    out = np.zeros((B, C, NX, NY), np.float32)
    for ci in range(8):
        pc = sched["per_core"][ci]
        slab = slabs[ci]
        for s, key in enumerate(pc["slot_rows"]):
            if key is None:
                continue
            b, xrow = key
            out[b, :, xrow, :] = slab[:, s * NY : (s + 1) * NY]
    return out


def kernel(x, camera2lidar_rots, camera2lidar_trans):
    from concourse.bass_utils import run_bass_kernel_spmd

    x = np.asarray(x, dtype=np.float32)
    rots = np.asarray(camera2lidar_rots, dtype=np.float32)
    trans = np.asarray(camera2lidar_trans, dtype=np.float32)
    sched = build_schedule(rots, trans)
    nc = build_program(sched)
    in_maps = make_in_maps(sched, x)
    res = run_bass_kernel_spmd(nc, in_maps, list(range(8)))
    slabs = [res.results[ci]["out"] for ci in range(8)]
    return assemble(slabs, sched)


# revision 4
# speedup vs baseline: 2.3470x; 1.1395x over previous
"""Trainium2 Bass kernel for BaseFisheyeLSSTransform (BEV pooling), v3.

Architecture (8-core SPMD, one program, per-core data tables):
- Host (index-only math): voxelize the frustum geometry on jax-cpu, find
  runs of kept points (consecutive source rows, same output x-row), encode
  them as indirect-DMA descriptors in two classes (L=1 single rows, L=4
  spans). Slots (b, x-row) are balanced across cores; per-(slot, class)
  descriptor counts are quantized and maxed over cores so all 8 cores share
  one instruction structure.
- Device: x is staged as one concatenated [2*566400, 80] fp16 tensor. Per
  instruction one indirect DMA gathers 128 descriptors into SBUF
  [128, L*80] fp16. Per (instruction, lane, slot) segment, a single fused
  DVE op builds M = (iota == vid) * invcnt in fp16 ([128, 360]); rows
  outside the segment (padding, other slots) carry vid = -1 so M is zero
  there and the fp16 matmul can contract over all 128 partitions without
  masking: PSUM[slot] += g_lane^T @ M. Closed slots are copied
  PSUM -> slab on the Scalar engine and flushed to DRAM in 16-slot blocks.
- Host assembles [2, 80, 360, 360] from the 8 slabs (pure unshard).
"""
import sys

sys.path.insert(0, "/opt/trn_rl_repo")

import numpy as np

B, N, C = 2, 4, 80
FH, FW, D = 40, 60, 59
NX, NY = 360, 360
PB = N * D * FH * FW  # 566400 rows per batch
P = 128
CLASSES = (2,)
QUANT = {2: 32}
FLUSH_WINDOWS = 16


# ---------------------------------------------------------------- host side


def _geometry(camera2lidar_rots, camera2lidar_trans):
    import jax
    import jax.numpy as jnp

    cpu = jax.devices("cpu")[0]
    with jax.default_device(cpu):
        DX = jnp.array([0.3, 0.3, 8.0], dtype=jnp.float32)
        ORIGIN = jnp.array([-54.0, -54.0, -5.0], dtype=jnp.float32)
        ds = jnp.arange(1.0, 60.0, 1.0, dtype=jnp.float32)
        az = jnp.linspace(-1.92, 1.92, FW, dtype=jnp.float32)
        el = jnp.linspace(-0.61, 0.61, FH, dtype=jnp.float32)
        d_, e_, a_ = ds[:, None, None], el[None, :, None], az[None, None, :]
        xs = d_ * jnp.cos(e_) * jnp.sin(a_)
        ys = jnp.broadcast_to(d_ * jnp.sin(e_), (D, FH, FW))
        zs = d_ * jnp.cos(e_) * jnp.cos(a_)
        fr = jnp.stack([xs, ys, zs], axis=-1)
        geom = jnp.einsum("bnij,dhwj->bndhwi", camera2lidar_rots, fr)
        geom = geom + camera2lidar_trans[:, :, None, None, None, :]
        coords = np.asarray(((geom - ORIGIN) / DX).astype(jnp.int32))
    kept = (
        (coords[..., 0] >= 0) & (coords[..., 0] < NX)
        & (coords[..., 1] >= 0) & (coords[..., 1] < NY)
        & (coords[..., 2] >= 0) & (coords[..., 2] < 1)
    )
    return coords, kept


def _build_runs(coords, kept):
    """rows[(b, xrow)] = {L: [(global_start, ys[L], ws[L])]}; ys=-1 pads."""
    Lmax = max(CLASSES)
    rows = {}
    for b in range(B):
        k = kept[b].reshape(-1)
        cx = coords[b, ..., 0].reshape(-1)
        cy = coords[b, ..., 1].reshape(-1)
        pts = np.flatnonzero(k)
        lin = cx[pts].astype(np.int64) * NY + cy[pts]
        cnt = np.bincount(lin, minlength=NX * NY)
        w_all = (1.0 / np.maximum(cnt, 1)).astype(np.float32)
        order = np.lexsort((pts, cx[pts]))
        sp = pts[order]
        sx = cx[pts][order]
        sy = cy[pts][order]
        sw = w_all[lin[order]]
        new = np.ones(sp.size, bool)
        new[1:] = (np.diff(sx) != 0) | (np.diff(sp) > Lmax)
        starts = np.flatnonzero(new)
        ends = np.append(starts[1:], sp.size)
        for s, e in zip(starts, ends):
            key = (b, int(sx[s]))
            if key not in rows:
                rows[key] = {L: [] for L in CLASSES}
            i = s
            while i < e:
                j = i
                base = int(sp[i])
                while j < e and int(sp[j]) - base < Lmax:
                    j += 1
                span = int(sp[j - 1]) - base + 1
                L = min(c for c in CLASSES if c >= span)
                gstart = b * PB + base
                gstart = min(gstart, 2 * PB - L)
                off = b * PB + base - gstart
                ys = np.full(L, -1.0, np.float32)
                ws = np.zeros(L, np.float32)
                for t in range(i, j):
                    ys[int(sp[t]) - base + off] = float(sy[t])
                    ws[int(sp[t]) - base + off] = float(sw[t])
                rows[key][L].append((gstart, ys, ws))
                i = j
    return rows


def _assign_slots(rows, n_cores=8):
    keys = sorted(rows, key=lambda k: -sum(len(v) for v in rows[k].values()))
    cores = [[] for _ in range(n_cores)]
    load = [0] * n_cores
    for k in keys:
        cost = sum(len(v) for v in rows[k].values())
        ci = min(range(n_cores), key=lambda i: load[i])
        cores[ci].append(k)
        load[ci] += cost
    return cores, load


def _ceil(a, b):
    return -(-a // b)


def build_schedule(camera2lidar_rots, camera2lidar_trans):
    coords, kept = _geometry(camera2lidar_rots, camera2lidar_trans)
    rows = _build_runs(coords, kept)
    cores, load = _assign_slots(rows)
    n_cores = len(cores)
    NSLOTS = max(len(c) for c in cores)

    nchunks = {}
    for cls in CLASSES:
        Q = QUANT[cls]
        for w in range(NSLOTS):
            m = 0
            for ci in range(n_cores):
                if w < len(cores[ci]):
                    m = max(m, _ceil(len(rows[cores[ci][w]][cls]), Q))
            nchunks[(cls, w)] = m

    instrs = []
    for cls in CLASSES:
        Q = QUANT[cls]
        npc = P // Q
        stream = []
        for w in range(NSLOTS):
            stream += [(w, q) for q in range(nchunks[(cls, w)])]
        for i0 in range(0, len(stream), npc):
            instrs.append(dict(cls=cls, chunks=stream[i0 : i0 + npc]))
    instrs.sort(key=lambda r: (min(c[0] for c in r["chunks"]),
                               r["cls"], r["chunks"][0][1]))
    NINSTR = len(instrs)

    # per-instruction chunk content per core
    content = []
    for rec in instrs:
        cls = rec["cls"]
        Q = QUANT[cls]
        per_chunk = []
        for (w, q) in rec["chunks"]:
            cores_dat = []
            for ci in range(n_cores):
                dlist = rows[cores[ci][w]][cls] if w < len(cores[ci]) else []
                chunk = dlist[q * Q : (q + 1) * Q]
                starts = np.zeros(Q, np.int64)
                ys = np.full((Q, cls), -1.0, np.float32)
                ws = np.zeros((Q, cls), np.float32)
                for j, (st, yy, ww) in enumerate(chunk):
                    starts[j] = st
                    ys[j] = yy
                    ws[j] = ww
                cores_dat.append((starts, ys, ws))
            per_chunk.append(cores_dat)
        content.append(per_chunk)

    # segments (uniform): (lane, k0, k1, w, col) — emitted if any core has
    # a real point; col indexes the vid/invpc tables
    NMBUILD = 0
    slot_last = {}
    slot_first = {}
    for ii, rec in enumerate(instrs):
        cls = rec["cls"]
        Q = QUANT[cls]
        segs = []
        for lane in range(cls):
            groups = []
            for k, (w, q) in enumerate(rec["chunks"]):
                if groups and groups[-1][0] == w and groups[-1][2] == k:
                    groups[-1][2] = k + 1
                else:
                    groups.append([w, k, k + 1])
            for (w, k0, k1) in groups:
                occ = any(
                    (content[ii][k][ci][1][:, lane] >= 0).any()
                    for k in range(k0, k1) for ci in range(n_cores)
                )
                if occ:
                    segs.append([lane, k0 * Q, k1 * Q, w, NMBUILD])
                    NMBUILD += 1
                    if w not in slot_first:
                        slot_first[w] = (ii, len(segs) - 1)
                    slot_last[w] = (ii, len(segs) - 1)
        rec["segments"] = segs

    # start/stop flags per segment; copies/flushes per instruction
    for ii, rec in enumerate(instrs):
        rec["copies_after"] = []
        rec["flushes"] = []
        for si, seg in enumerate(rec["segments"]):
            w = seg[3]
            seg.append(slot_first[w] == (ii, si))
            seg.append(slot_last[w] == (ii, si))
    for w, (ii, si) in slot_last.items():
        instrs[ii]["copies_after"].append(w)
    nblocks = _ceil(NSLOTS, FLUSH_WINDOWS)
    for blk in range(nblocks):
        ws = [w for w in range(blk * FLUSH_WINDOWS,
                               min((blk + 1) * FLUSH_WINDOWS, NSLOTS))
              if w in slot_last]
        pos = max(slot_last[w][0] for w in ws) if ws else 0
        instrs[pos]["flushes"].append(blk)

    # per-core tables
    per_core = []
    for ci in range(n_cores):
        desc = np.zeros((P, NINSTR), np.int32)
        vid = np.full((P, max(NMBUILD, 1)), -1.0, np.float32)
        invpc = np.zeros((P, max(NMBUILD, 1)), np.float32)
        for ii, rec in enumerate(instrs):
            cls = rec["cls"]
            Q = QUANT[cls]
            for k in range(len(rec["chunks"])):
                starts, ys, ws = content[ii][k][ci]
                desc[k * Q : (k + 1) * Q, ii] = starts
            for seg in rec["segments"]:
                lane, lo, hi, w, col = seg[:5]
                for k in range(lo // Q, hi // Q):
                    starts, ys, ws = content[ii][k][ci]
                    vid[k * Q : (k + 1) * Q, col] = ys[:, lane]
                    invpc[k * Q : (k + 1) * Q, col] = ws[:, lane]
        slot_rows = [cores[ci][w] if w < len(cores[ci]) else None
                     for w in range(NSLOTS)]
        per_core.append(dict(desc=desc, vid=vid, invpc=invpc,
                             slot_rows=slot_rows))

    return dict(instrs=instrs, NINSTR=NINSTR, NMBUILD=NMBUILD,
                NSLOTS=NSLOTS, per_core=per_core, load=load,
                nblocks=nblocks)


# ---------------------------------------------------------------- device


def build_program(sched):
    import concourse.bacc as bacc
    import concourse.bass as bass
    import concourse.mybir as mybir
    import concourse.tile as tile

    f32, f16 = mybir.dt.float32, mybir.dt.float16
    i32 = mybir.dt.int32
    NINSTR, NMBUILD = sched["NINSTR"], sched["NMBUILD"]
    NSLOTS = sched["NSLOTS"]

    nc = bacc.Bacc(None)
    xb = nc.declare_dram_parameter("xb", [2 * PB, C], f16, isOutput=False)
    desc_d = nc.declare_dram_parameter("desc", [P, NINSTR], i32,
                                       isOutput=False)
    vid_d = nc.declare_dram_parameter("vid", [P, NMBUILD], f32,
                                      isOutput=False)
    invpc_d = nc.declare_dram_parameter("invpc", [P, NMBUILD], f32,
                                        isOutput=False)
    iota_d = nc.declare_dram_parameter("iota", [P, NY], f16, isOutput=False)
    out_d = nc.declare_dram_parameter("out", [C, NSLOTS * NY], f32,
                                      isOutput=True)

    with tile.TileContext(nc) as tc:
        with (
            tc.tile_pool(name="const", bufs=1) as cpool,
            tc.tile_pool(name="g2", bufs=12) as g2pool,
            tc.tile_pool(name="m", bufs=8) as mpool,
            tc.tile_pool(name="psum", bufs=8, space="PSUM") as ppool,
            tc.tile_pool(name="slab", bufs=3) as slabpool,
        ):
            desc_t = cpool.tile([P, NINSTR], i32)
            vid_t = cpool.tile([P, NMBUILD], f32)
            invpc_t = cpool.tile([P, NMBUILD], f32)
            iota_t = cpool.tile([P, NY], f16)
            nc.sync.dma_start(out=desc_t[:], in_=desc_d[:])
            nc.sync.dma_start(out=vid_t[:], in_=vid_d[:])
            nc.sync.dma_start(out=invpc_t[:], in_=invpc_d[:])
            nc.sync.dma_start(out=iota_t[:], in_=iota_d[:])

            wtiles = {}
            slabs = {}
            for ii, rec in enumerate(sched["instrs"]):
                L = rec["cls"]
                g = g2pool.tile([P, L * C], f16, tag=f"g{L}")
                nc.gpsimd.indirect_dma_start(
                    out=g[:],
                    out_offset=None,
                    in_=xb[:],
                    in_offset=bass.IndirectOffsetOnAxis(
                        ap=desc_t[:, ii : ii + 1], axis=0
                    ),
                )
                for seg in rec["segments"]:
                    lane, lo, hi, w, col, st, sp_ = seg
                    M = mpool.tile([P, NY], f16, tag="m")
                    nc.vector.tensor_scalar(
                        out=M[:],
                        in0=iota_t[:],
                        scalar1=vid_t[:, col : col + 1],
                        scalar2=invpc_t[:, col : col + 1],
                        op0=mybir.AluOpType.is_equal,
                        op1=mybir.AluOpType.mult,
                    )
                    if st:
                        wtiles[w] = ppool.tile([C, NY], f32, tag="w",
                                               name=f"w{w}")
                    nc.tensor.matmul(
                        wtiles[w][:],
                        g[:, lane * C : (lane + 1) * C],
                        M[:],
                        start=st,
                        stop=sp_,
                        skip_group_check=True,
                    )
                for w in rec["copies_after"]:
                    blk = w // FLUSH_WINDOWS
                    if blk not in slabs:
                        slabs[blk] = slabpool.tile(
                            [C, FLUSH_WINDOWS * NY], f32, tag="slab",
                            name=f"slab{blk}",
                        )
                    off = w % FLUSH_WINDOWS
                    nc.scalar.activation(
                        out=slabs[blk][:, off * NY : (off + 1) * NY],
                        in_=wtiles.pop(w)[:],
                        func=mybir.ActivationFunctionType.Copy,
                    )
                for blk in rec["flushes"]:
                    w0 = blk * FLUSH_WINDOWS
                    w1 = min(w0 + FLUSH_WINDOWS, NSLOTS)
                    nc.sync.dma_start(
                        out=out_d[:, w0 * NY : w1 * NY],
                        in_=slabs.pop(blk)[:, : (w1 - w0) * NY],
                    )
    nc.compile()
    return nc


def make_in_maps(sched, x):
    xcat = np.ascontiguousarray(
        x.reshape(2 * PB, C)).astype(np.float16)
    iota = np.broadcast_to(
        np.arange(NY, dtype=np.float16)[None, :], (P, NY)
    ).copy()
    in_maps = []
    for ci in range(8):
        pc = sched["per_core"][ci]
        in_maps.append(
            {
                "xb": xcat,
                "desc": pc["desc"],
                "vid": pc["vid"],
                "invpc": pc["invpc"],
                "iota": iota,
            }
        )
    return in_maps


def assemble(slabs, sched):
    out = np.zeros((B, C, NX, NY), np.float32)
    for ci in range(8):
        pc = sched["per_core"][ci]
        slab = slabs[ci]
        for s, key in enumerate(pc["slot_rows"]):
            if key is None:
                continue
            b, xrow = key
            out[b, :, xrow, :] = slab[:, s * NY : (s + 1) * NY]
    return out


def kernel(x, camera2lidar_rots, camera2lidar_trans):
    from concourse.bass_utils import run_bass_kernel_spmd

    x = np.asarray(x, dtype=np.float32)
    rots = np.asarray(camera2lidar_rots, dtype=np.float32)
    trans = np.asarray(camera2lidar_trans, dtype=np.float32)
    sched = build_schedule(rots, trans)
    nc = build_program(sched)
    in_maps = make_in_maps(sched, x)
    res = run_bass_kernel_spmd(nc, in_maps, list(range(8)))
    slabs = [res.results[ci]["out"] for ci in range(8)]
    return assemble(slabs, sched)
